# revision 5
# baseline (speedup 1.0000x reference)
"""Trainium2 Bass kernel for nn_Attention_80779744903968.

Reference computation (B=32, T=512, S=1024, H=1024):
    z      = q @ W_in.T                  [B,T,H]
    scores = z @ enc_b.T                 [B,T,S]   (enc input is [S,B,H])
    p      = softmax(scores, axis=-1)    (the scores==0 -> -inf fill is a
                                          numerical no-op: row maxes are ~120,
                                          exp(0-max) == 0 in fp32)
    c      = p @ enc_b                   [B,T,H]
    out    = tanh([c, q] @ W_out.T + b)  [B,T,H]

Sharding: data-parallel over B across 8 cores (4 batches per core).
W_in / W_out replicated.

Precision strategy (PE matmuls):
  - z and scores need near-fp32 logits: the softmax is near-one-hot
    (logit std ~37) with near-tied rows. Both are computed as an fp16
    hi/lo split: x*y ~= xh*yh (fp16 main pass, fp32 PSUM accumulation)
    + (xh*yl + xl*yh) correction passes in fp8(e4m3) with
    perf_mode=DoubleRow at 2 contraction-tiles per matmul; correction
    operands are pre-scaled by powers of 2 so both corr products share one
    PSUM scale. (Cheaper variants - dropping either correction term, or
    fp32r mains - measure > 2e-2 final error; this scheme is minimal.)
  - downstream (p, enc, c, q, W_out) runs in plain fp16: p is in [0,1] and
    c/out magnitudes are O(1), so fp16's 2^-11 relative error is plenty.

Schedule notes (from perfetto/NTFF analysis):
  - every dma_start costs ~0.7-1.3us of issue time on the issuing engine's
    queue. Only SP and Activation have hardware DGE on TRN2; GPSIMD is
    software DGE (~87GB/s) so it only carries tensors needed late (en, wo).
    The z-correction fp8 weights ride the Activation queue so batch-0's
    corrections never wait on the SP queue's 16 wh/qh chunk transfers.
  - eviction chains are spread across DVE/Act/Pool so no single engine
    stalls the PE: z-corr evict = stt+mul (DVE), f16 round (Act),
    sub + f8 cast (Pool); scores evict = copy (Act) + stt (DVE).
  - the p-transposes of tile tt-1 are interleaved behind the scores
    matmuls of tile tt; the softmax tail of the last tile is covered by
    prefilling the out-projection's q-part matmuls into the then-idle
    correction PSUM banks.
  - softmax row-max is reduced per 512-half (overlaps the next half's
    matmuls) and exp produces its row-sum via accum_out, shortening the
    critical chain to exp+recip+mul.

All transposes (q -> [H,T], enc -> [H,S] per batch) are done on the host so
every device-side DMA is a contiguous natural-layout load; only the softmax
output p is transposed on-device (PE transpose-mode, fp16, 128x128 tiles).
"""
import os
import sys

import numpy as np

sys.path.insert(0, "/opt/trn_rl_repo")

import ml_dtypes  # noqa: E402

import concourse.bass as bass  # noqa: E402
import concourse.tile as tile  # noqa: E402
from concourse import bacc, mybir  # noqa: E402
from concourse.bass_utils import run_bass_kernel_spmd  # noqa: E402
from concourse.masks import make_identity  # noqa: E402

B, T, S, H = 32, 512, 1024, 1024
NCORES = 8
BL = B // NCORES  # batches per core
HT = H // 128     # h/i/k tiles per 1024
TT = T // 128     # t tiles
ST = S // 128     # s tiles
F16 = mybir.dt.float16
F32 = mybir.dt.float32
F8 = mybir.dt.float8e4
DR = mybir.MatmulPerfMode.DoubleRow

# power-of-2 scales for fp8 correction operands (products must share scale)
SC_WH, SC_WL, SC_QH, SC_QL = 2.0**4, 2.0**15, 1.0, 2.0**11   # z corr: 2^15
SC_ZL, SC_EH, SC_EL = 2.0**12, 1.0, 2.0**12                  # s corr: 2^12

_CACHE = {}


def _build(has_bias):
    nc = bacc.Bacc("TRN2", target_bir_lowering=False, debug=False,
                   num_devices=NCORES)

    def din(name, shape, dt=F16):
        return nc.dram_tensor(name, shape, dt, kind="ExternalInput").ap()

    qh_d = din("qh", [BL, H, T])
    eh_d = din("eh", [BL, H, S])
    en_d = din("en", [BL, S, H])
    wh_d = din("wh", [H, H])
    wo_d = din("wo", [2 * H, H])
    bias_d = din("bias", [128, H], F32)
    ql8_d = din("ql8", [BL, H, T], F8)
    el8_d = din("el8", [BL, H, S], F8)
    qh8_d = din("qh8", [BL, H, T], F8)
    eh8_d = din("eh8", [BL, H, S], F8)
    wh8_d = din("wh8", [H, H], F8)
    wl8_d = din("wl8", [H, H], F8)
    out_d = nc.dram_tensor("out", [BL, T, H], F32, kind="ExternalOutput").ap()

    with tile.TileContext(nc) as tc:
        with (
            tc.tile_pool(name="weights", bufs=1) as wp,
            tc.tile_pool(name="qin", bufs=2) as qp,
            tc.tile_pool(name="ein", bufs=1) as ep,
            tc.tile_pool(name="enin", bufs=1) as enp,
            tc.tile_pool(name="zbuf", bufs=1) as zp,
            tc.tile_pool(name="scratch", bufs=1) as scrp,
            tc.tile_pool(name="sctile", bufs=2) as scrp2,
            tc.tile_pool(name="scores", bufs=1) as scp,
            tc.tile_pool(name="pbuf", bufs=3) as pp,
            tc.tile_pool(name="ptbuf", bufs=1) as ptp,
            tc.tile_pool(name="ctbuf", bufs=1) as ctp,
            tc.tile_pool(name="ostage", bufs=2) as op,
            tc.tile_pool(name="stats", bufs=8) as stp,
            tc.tile_pool(name="psmm", bufs=4, space="PSUM") as psmm,
            tc.tile_pool(name="psc", bufs=2, space="PSUM") as psc,
            tc.tile_pool(name="pstr", bufs=2, space="PSUM") as pstr,
        ):
            # --- resident weights / constants ---
            wh_t = wp.tile([128, HT, H], F16)
            wh_r = wh_d.rearrange("(ht p) i -> p ht i", p=128)
            qh_first = qp.tile([128, HT, T], F16, tag="qh")
            qh_r = qh_d[0].rearrange("(ht p) t -> p ht t", p=128)
            for ht in range(HT):
                nc.sync.dma_start(wh_t[:, ht, :], wh_r[:, ht, :])
                nc.sync.dma_start(qh_first[:, ht, :], qh_r[:, ht, :])
            qh8_first = qp.tile([128, HT, T], F8, tag="qh8", bufs=1)
            nc.sync.dma_start(
                qh8_first[:], qh8_d[0].rearrange("(ht p) t -> p ht t", p=128))
            ql8_first = qp.tile([128, HT, T], F8, tag="ql8", bufs=1)
            nc.sync.dma_start(
                ql8_first[:], ql8_d[0].rearrange("(ht p) t -> p ht t", p=128))
            bias_t = wp.tile([128, H], F32)
            nc.sync.dma_start(bias_t[:], bias_d)

            wl8_t = wp.tile([128, HT, H], F8)
            nc.scalar.dma_start(
                wl8_t[:], wl8_d.rearrange("(ht p) i -> p ht i", p=128))
            wh8_t = wp.tile([128, HT, H], F8)
            nc.scalar.dma_start(
                wh8_t[:], wh8_d.rearrange("(ht p) i -> p ht i", p=128))
            ident = wp.tile([128, 128], F16)
            make_identity(nc, ident[:])
            wo_t = None

            def load_q(b):
                qh_t = qp.tile([128, HT, T], F16, tag="qh")
                nc.sync.dma_start(
                    qh_t[:], qh_d[b].rearrange("(ht p) t -> p ht t", p=128))
                qh8_t = qp.tile([128, HT, T], F8, tag="qh8", bufs=1)
                nc.sync.dma_start(
                    qh8_t[:], qh8_d[b].rearrange("(ht p) t -> p ht t", p=128))
                ql8_t = qp.tile([128, HT, T], F8, tag="ql8", bufs=1)
                nc.sync.dma_start(
                    ql8_t[:], ql8_d[b].rearrange("(ht p) t -> p ht t", p=128))
                return qh_t, qh8_t, ql8_t

            q_next = (qh_first, qh8_first, ql8_first)

            for b in range(BL):
                qh_t, qh8_t, ql8_t = q_next
                eh_t = ep.tile([128, HT, S], F16, tag="eh")
                nc.scalar.dma_start(
                    eh_t[:], eh_d[b].rearrange("(it p) s -> p it s", p=128))
                eh8_t = ep.tile([128, HT, S], F8, tag="eh8")
                nc.scalar.dma_start(
                    eh8_t[:], eh8_d[b].rearrange("(it p) s -> p it s", p=128))
                el8_t = ep.tile([128, HT, S], F8, tag="el8")
                nc.scalar.dma_start(
                    el8_t[:], el8_d[b].rearrange("(it p) s -> p it s", p=128))
                en_t = enp.tile([128, ST, H], F16, tag="en")
                nc.gpsimd.dma_start(
                    en_t[:], en_d[b].rearrange("(st p) k -> p st k", p=128))
                if wo_t is None:
                    wo_t = wp.tile([128, 2 * HT, H], F16)
                    nc.gpsimd.dma_start(
                        wo_t[:], wo_d.rearrange("(kt p) h -> p kt h", p=128))

                # --- zT = W_inT.T @ qT: fp16 main + fp8 DR corrections,
                # staged through an f32 combine buffer. b0 runs pass-major
                # (mains need only wh+qh; corrections wait on fp8 DMAs);
                # later batches interleave so the eviction chains spread.
                comb_t = scrp.tile([128, HT, T], F32, tag="comb")

                def z_main(it):
                    zps = psmm.tile([128, T], F32, tag="mm")
                    for ht in range(HT):
                        nc.tensor.matmul(
                            zps[:],
                            wh_t[:, ht, it * 128:(it + 1) * 128],
                            qh_t[:, ht, :],
                            start=(ht == 0), stop=(ht == HT - 1))
                    nc.vector.tensor_copy(comb_t[:, it, :], zps[:])

                def corr_mms(out_ps, ops, it_lo, it_n, sc_lo, sc_n):
                    j = 0
                    for lhs, rhs in ops:
                        for k in range(HT // 2):
                            nc.tensor.matmul(
                                out_ps[:],
                                lhs[:, 2 * k:2 * k + 2, it_lo:it_lo + it_n],
                                rhs[:, 2 * k:2 * k + 2, sc_lo:sc_lo + sc_n],
                                start=(j == 0), stop=(j == HT - 1),
                                perf_mode=DR)
                            j += 1

                def z_corr(it):
                    zcorr = psc.tile([128, T], F32, tag="mmc")
                    corr_mms(zcorr, ((wl8_t, qh8_t), (wh8_t, ql8_t)),
                             it * 128, 128, 0, T)
                    comb = comb_t[:, it, :]
                    nc.vector.scalar_tensor_tensor(
                        out=comb, in0=zcorr[:],
                        scalar=1.0 / (SC_WH * SC_QL), in1=comb,
                        op0=mybir.AluOpType.mult, op1=mybir.AluOpType.add)
                    nc.scalar.activation(
                        out=zh_t[:, it, :], in_=comb,
                        func=mybir.ActivationFunctionType.Copy)
                    zl_tmp = scrp2.tile([128, T], F16, tag="zltmp")
                    nc.gpsimd.tensor_sub(zl_tmp[:], comb, zh_t[:, it, :])
                    nc.vector.tensor_scalar_mul(
                        zl8_t[:, it, :], zl_tmp[:], SC_ZL)
                    nc.gpsimd.tensor_copy(zh8_t[:, it, :], zh_t[:, it, :])

                zh_t = zp.tile([128, HT, T], F16, tag="zh")
                zh8_t = zp.tile([128, HT, T], F8, tag="zh8")
                zl8_t = zp.tile([128, HT, T], F8, tag="zl8")
                if b == 0:
                    for it in range(HT):
                        z_main(it)
                    for it in range(HT):
                        z_corr(it)
                else:
                    for it in range(HT):
                        z_main(it)
                        z_corr(it)

                # --- scores + softmax -> p; transposes interleaved ---
                pt_t = ptp.tile([128, ST, T], F16, tag="pt")
                p_tiles = []

                def transposes(tt):
                    for st in range(ST):
                        tps = pstr.tile([128, 128], F16, tag="tr")
                        nc.tensor.transpose(
                            tps[:], p_tiles[tt][:, st * 128:(st + 1) * 128],
                            ident[:])
                        nc.vector.tensor_copy(
                            pt_t[:, st, tt * 128:(tt + 1) * 128], tps[:])

                def scores_half(sc_t, tt, sc, nmax):
                    sps = psmm.tile([128, 512], F32, tag="mm")
                    for it in range(HT):
                        nc.tensor.matmul(
                            sps[:],
                            zh_t[:, it, tt * 128:(tt + 1) * 128],
                            eh_t[:, it, sc * 512:(sc + 1) * 512],
                            start=(it == 0), stop=(it == HT - 1))
                    scorr = psc.tile([128, 512], F32, tag="mmc")
                    corr_mms(scorr, ((zl8_t, eh8_t), (zh8_t, el8_t)),
                             tt * 128, 128, sc * 512, 512)
                    chunk = sc_t[:, sc * 512:(sc + 1) * 512]
                    nc.scalar.activation(
                        out=chunk, in_=sps[:],
                        func=mybir.ActivationFunctionType.Copy)
                    nc.vector.scalar_tensor_tensor(
                        out=chunk, in0=scorr[:],
                        scalar=1.0 / (SC_ZL * SC_EH), in1=chunk,
                        op0=mybir.AluOpType.mult,
                        op1=mybir.AluOpType.add)
                    # row-max of this half (negated) overlaps the next half
                    nc.vector.reduce_max(out=nmax[:], in_=chunk,
                                         axis=mybir.AxisListType.X,
                                         negate=True)

                def softmax(sc_t, nm0, nm1):
                    negmax = stp.tile([128, 1], F32, tag="nm")
                    nc.vector.tensor_tensor(
                        out=negmax[:], in0=nm0[:], in1=nm1[:],
                        op=mybir.AluOpType.min)
                    p_t = pp.tile([128, S], F16, tag="p")
                    ssum = stp.tile([128, 1], F32, tag="ss")
                    nc.scalar.activation(
                        out=p_t[:], in_=sc_t[:],
                        func=mybir.ActivationFunctionType.Exp,
                        bias=negmax[:], scale=1.0, accum_out=ssum[:])
                    rsum = stp.tile([128, 1], F32, tag="rs")
                    nc.vector.reciprocal(rsum[:], ssum[:])
                    nc.vector.tensor_scalar_mul(p_t[:], p_t[:], rsum[:])
                    p_tiles.append(p_t)

                for tt in range(TT):
                    sc_t = scp.tile([128, S], F32, tag="sc")
                    nm0 = stp.tile([128, 1], F32, tag="nm0")
                    nm1 = stp.tile([128, 1], F32, tag="nm1")
                    scores_half(sc_t, tt, 0, nm0)
                    scores_half(sc_t, tt, 1, nm1)
                    softmax(sc_t, nm0, nm1)
                    if tt > 0:
                        transposes(tt - 1)

                # prefill the q-part of the out projection for tt=0 into the
                # (now idle) correction PSUM banks; covers the last softmax
                # chain + transposes(3) latency with useful PE work.
                oq = []
                for hc in range(2):
                    ops = psc.tile([128, 512], F32, tag="mmc")
                    for ht in range(HT):
                        nc.tensor.matmul(
                            ops[:],
                            qh_t[:, ht, 0:128],
                            wo_t[:, HT + ht, hc * 512:(hc + 1) * 512],
                            start=(ht == 0), stop=False)
                    oq.append(ops)
                transposes(TT - 1)

                # prefetch next batch's q tensors on SP now, BEFORE the out
                # stores occupy the queue.
                if b + 1 < BL:
                    q_next = load_q(b + 1)

                # --- cT = enc_nat.T @ pT -> [k, t] f16 ---
                ct_t = ctp.tile([128, HT, T], F16, tag="ct")
                for kt in range(HT):
                    cps = psmm.tile([128, T], F32, tag="mm")
                    for st in range(ST):
                        nc.tensor.matmul(
                            cps[:],
                            en_t[:, st, kt * 128:(kt + 1) * 128],
                            pt_t[:, st, :],
                            start=(st == 0), stop=(st == ST - 1))
                    nc.scalar.activation(
                        out=ct_t[:, kt, :], in_=cps[:],
                        func=mybir.ActivationFunctionType.Copy)

                # --- out = tanh(cT.T @ WcT + qT.T @ WqT + b) ---
                for tt in range(TT):
                    for hc in range(2):
                        if tt == 0:
                            ops = oq[hc]
                        else:
                            ops = psmm.tile([128, 512], F32, tag="mm")
                            for ht in range(HT):
                                nc.tensor.matmul(
                                    ops[:],
                                    qh_t[:, ht, tt * 128:(tt + 1) * 128],
                                    wo_t[:, HT + ht, hc * 512:(hc + 1) * 512],
                                    start=(ht == 0), stop=False)
                        for kt in range(HT):
                            nc.tensor.matmul(
                                ops[:],
                                ct_t[:, kt, tt * 128:(tt + 1) * 128],
                                wo_t[:, kt, hc * 512:(hc + 1) * 512],
                                start=False, stop=(kt == HT - 1))
                        ost = op.tile([128, 512], F32, tag="os")
                        if has_bias:
                            nc.vector.tensor_add(
                                ost[:], ops[:],
                                bias_t[:, hc * 512:(hc + 1) * 512])
                            nc.scalar.activation(
                                out=ost[:], in_=ost[:],
                                func=mybir.ActivationFunctionType.Tanh)
                        else:
                            nc.scalar.activation(
                                out=ost[:], in_=ops[:],
                                func=mybir.ActivationFunctionType.Tanh)
                        nc.sync.dma_start(
                            out_d[b, tt * 128:(tt + 1) * 128,
                                  hc * 512:(hc + 1) * 512],
                            ost[:])

    nc.compile()
    return nc


def _get_nc(has_bias):
    key = ("nc", has_bias)
    if key not in _CACHE:
        _CACHE[key] = _build(has_bias)
    return _CACHE[key]


def _split16(x):
    hi = x.astype(np.float16)
    lo = (x - hi.astype(np.float32)).astype(np.float32)
    return hi, lo


def _f8(x, scale):
    return (np.asarray(x, np.float32) * np.float32(scale)).astype(
        ml_dtypes.float8_e4m3)


def kernel(query, encoder_outputs, src_lengths, W_in, W_out, b_out):
    query = np.asarray(query, np.float32)
    enc = np.asarray(encoder_outputs, np.float32)
    W_in = np.asarray(W_in, np.float32)
    W_out = np.asarray(W_out, np.float32)
    b_out = np.asarray(b_out, np.float32)

    # host-side layout prep (transposes + fp16 hi/lo splits)
    qT = np.ascontiguousarray(query.transpose(0, 2, 1))        # [B, H, T]
    qh, ql = _split16(qT)
    encT = np.ascontiguousarray(enc.transpose(1, 2, 0))        # [B, H, S]
    eh, el = _split16(encT)
    en = np.ascontiguousarray(enc.transpose(1, 0, 2)).astype(np.float16)
    whf, wlf = _split16(np.ascontiguousarray(W_in.T))          # [H(h), H(i)]
    wo = np.ascontiguousarray(W_out.T).astype(np.float16)      # [2H, H]
    bias = np.ascontiguousarray(
        np.broadcast_to(b_out[None, :], (128, H)), np.float32)

    common = {
        "wh": whf, "wo": wo, "bias": bias,
        "wh8": _f8(whf.astype(np.float32), SC_WH),
        "wl8": _f8(wlf, SC_WL),
    }

    in_maps = []
    for c in range(NCORES):
        sl = slice(c * BL, (c + 1) * BL)
        m = {
            "qh": np.ascontiguousarray(qh[sl]),
            "eh": np.ascontiguousarray(eh[sl]),
            "en": np.ascontiguousarray(en[sl]),
            "qh8": _f8(qh[sl].astype(np.float32), SC_QH),
            "ql8": _f8(ql[sl], SC_QL),
            "eh8": _f8(eh[sl].astype(np.float32), SC_EH),
            "el8": _f8(el[sl], SC_EL),
            **common,
        }
        in_maps.append(m)

    nc = _get_nc(bool(np.any(b_out)))
    trace = bool(int(os.environ.get("KERNEL_TRACE", "0")))
    res = run_bass_kernel_spmd(nc, in_maps, core_ids=list(range(NCORES)),
                               trace=trace)
    if trace:
        _CACHE["last_exec_time_ns"] = res.exec_time_ns
        _CACHE["last_results"] = res
    out = np.concatenate([r["out"] for r in res.results], axis=0)
    return out


# revision 7
# speedup vs baseline: 1.3957x; 1.3957x over previous
"""Trainium2 Bass kernel for nn_Attention_80779744903968.

Reference computation (B=32, T=512, S=1024, H=1024):
    z      = q @ W_in.T                  [B,T,H]
    scores = z @ enc_b.T                 [B,T,S]   (enc input is [S,B,H])
    p      = softmax(scores, axis=-1)    (the scores==0 -> -inf fill is a
                                          numerical no-op: row maxes are ~120,
                                          exp(0-max) == 0 in fp32)
    c      = p @ enc_b                   [B,T,H]
    out    = tanh([c, q] @ W_out.T + b)  [B,T,H]

Sharding: data-parallel over B across 8 cores (4 batches per core).
W_in / W_out replicated.

Precision strategy (PE matmuls):
  - z and scores need near-fp32 logits: the softmax is near-one-hot
    (logit std ~37) with near-tied rows. Both are computed as an fp16
    hi/lo split: x*y ~= xh*yh (fp16 main pass, fp32 PSUM accumulation)
    + (xh*yl + xl*yh) correction passes in fp8(e4m3) with
    perf_mode=DoubleRow at 2 contraction-tiles per matmul; correction
    operands are pre-scaled by powers of 2 so both corr products share one
    PSUM scale. (Cheaper variants - dropping either correction term, or
    fp32r mains - measure > 2e-2 final error; this scheme is minimal.)
  - downstream (p, enc, c, q, W_out) runs in plain fp16: p is in [0,1] and
    c/out magnitudes are O(1), so fp16's 2^-11 relative error is plenty.

Schedule notes (from perfetto/NTFF analysis):
  - every dma_start costs ~0.7-1.3us of issue time on the issuing engine's
    queue. Only SP and Activation have hardware DGE on TRN2; GPSIMD is
    software DGE (~87GB/s) so it only carries tensors needed late (en, wo).
    The z-correction fp8 weights ride the Activation queue so batch-0's
    corrections never wait on the SP queue's 16 wh/qh chunk transfers.
  - eviction chains are spread across DVE/Act/Pool so no single engine
    stalls the PE: z-corr evict = stt+mul (DVE), f16 round (Act),
    sub + f8 cast (Pool); scores evict = copy (Act) + stt (DVE).
  - the p-transposes of tile tt-1 are interleaved behind the scores
    matmuls of tile tt; the softmax tail of the last tile is covered by
    prefilling the out-projection's q-part matmuls into the then-idle
    correction PSUM banks.
  - softmax row-max is reduced per 512-half (overlaps the next half's
    matmuls) and exp produces its row-sum via accum_out, shortening the
    critical chain to exp+recip+mul.

All transposes (q -> [H,T], enc -> [H,S] per batch) are done on the host so
every device-side DMA is a contiguous natural-layout load; only the softmax
output p is transposed on-device (PE transpose-mode, fp16, 128x128 tiles).
"""
import os
import sys

import numpy as np

sys.path.insert(0, "/opt/trn_rl_repo")

import ml_dtypes  # noqa: E402

import concourse.bass as bass  # noqa: E402
import concourse.tile as tile  # noqa: E402
from concourse import bacc, mybir  # noqa: E402
from concourse.bass_utils import run_bass_kernel_spmd  # noqa: E402
from concourse.masks import make_identity  # noqa: E402

B, T, S, H = 32, 512, 1024, 1024
NCORES = 8
BL = B // NCORES  # batches per core
HT = H // 128     # h/i/k tiles per 1024
TT = T // 128     # t tiles
ST = S // 128     # s tiles
F16 = mybir.dt.float16
F32 = mybir.dt.float32
F8 = mybir.dt.float8e4
DR = mybir.MatmulPerfMode.DoubleRow

# power-of-2 scales for fp8 correction operands (products must share scale)
SC_WH, SC_WL, SC_QH, SC_QL = 2.0**4, 2.0**15, 1.0, 2.0**11   # z corr: 2^15
SC_ZL, SC_EH, SC_EL = 2.0**12, 1.0, 2.0**12                  # s corr: 2^12

_CACHE = {}


def _build(has_bias):
    HAS_BIAS = has_bias
    nc = bacc.Bacc("TRN2", target_bir_lowering=False, debug=False,
                   num_devices=NCORES)

    def din(name, shape, dt=F16):
        return nc.dram_tensor(name, shape, dt, kind="ExternalInput").ap()

    qh_d = din("qh", [BL, H, T])
    eh_d = din("eh", [BL, H, S])
    en_d = din("en", [BL, S, H])
    wh_d = din("wh", [H, H])
    wo_d = din("wo", [2 * H, H])
    bias_d = din("bias", [128, H], F32)
    ql8_d = din("ql8", [BL, H, T], F8)
    el8_d = din("el8", [BL, H, S], F8)
    qh8_d = din("qh8", [BL, H, T], F8)
    eh8_d = din("eh8", [BL, H, S], F8)
    wh8_d = din("wh8", [H, H], F8)
    wl8_d = din("wl8", [H, H], F8)
    out_d = nc.dram_tensor("out", [BL, T, H], F32, kind="ExternalOutput").ap()

    with tile.TileContext(nc) as tc:
        with (
            tc.tile_pool(name="weights", bufs=1) as wp,
            tc.tile_pool(name="qin", bufs=2) as qp,
            tc.tile_pool(name="ein", bufs=1) as ep,
            tc.tile_pool(name="enin", bufs=1) as enp,
            tc.tile_pool(name="zbuf", bufs=1) as zp,
            tc.tile_pool(name="scratch", bufs=1) as scrp,
            tc.tile_pool(name="sctile", bufs=2) as scrp2,
            tc.tile_pool(name="scores", bufs=1) as scp,
            tc.tile_pool(name="pbuf", bufs=3) as pp,
            tc.tile_pool(name="ptbuf", bufs=1) as ptp,
            tc.tile_pool(name="ctbuf", bufs=1) as ctp,
            tc.tile_pool(name="ostage", bufs=2) as op,
            tc.tile_pool(name="stats", bufs=8) as stp,
            tc.tile_pool(name="psmm", bufs=4, space="PSUM") as psmm,
            tc.tile_pool(name="psc", bufs=2, space="PSUM") as psc,
            tc.tile_pool(name="pstr", bufs=2, space="PSUM") as pstr,
        ):
            # --- resident weights / constants ---
            wh_t = wp.tile([128, HT, H], F16)
            wh_r = wh_d.rearrange("(ht p) i -> p ht i", p=128)
            qh_first = qp.tile([128, HT, T], F16, tag="qh")
            qh_r = qh_d[0].rearrange("(ht p) t -> p ht t", p=128)
            for ht in range(HT):
                nc.sync.dma_start(wh_t[:, ht, :], wh_r[:, ht, :])
                nc.sync.dma_start(qh_first[:, ht, :], qh_r[:, ht, :])
            wl8_t = wp.tile([128, HT, H], F8)
            nc.sync.dma_start(
                wl8_t[:], wl8_d.rearrange("(ht p) i -> p ht i", p=128))
            ql8_first = qp.tile([128, HT, T], F8, tag="ql8", bufs=1)
            nc.sync.dma_start(
                ql8_first[:], ql8_d[0].rearrange("(ht p) t -> p ht t", p=128))
            # b0's wh8/qh8 are derived on ACT from the just-landed fp16
            # tensors instead of DMA'd: trims 1.5MB off the startup-critical
            # SP queue so the first z-corrections never stall.
            wh8_t = wp.tile([128, HT, H], F8)
            qh8_first = qp.tile([128, HT, T], F8, tag="qh8", bufs=1)
            for ht in range(HT):
                nc.scalar.activation(
                    out=wh8_t[:, ht, :], in_=wh_t[:, ht, :],
                    func=mybir.ActivationFunctionType.Copy, scale=SC_WH)
                nc.scalar.activation(
                    out=qh8_first[:, ht, :], in_=qh_first[:, ht, :],
                    func=mybir.ActivationFunctionType.Copy, scale=SC_QH)
            if HAS_BIAS:
                bias_t = wp.tile([128, H], F32)
                nc.sync.dma_start(bias_t[:], bias_d)
            ident = wp.tile([128, 128], F16)
            make_identity(nc, ident[:])
            wo_t = None

            def load_q(b):
                qh_t = qp.tile([128, HT, T], F16, tag="qh")
                nc.sync.dma_start(
                    qh_t[:], qh_d[b].rearrange("(ht p) t -> p ht t", p=128))
                qh8_t = qp.tile([128, HT, T], F8, tag="qh8", bufs=1)
                nc.sync.dma_start(
                    qh8_t[:], qh8_d[b].rearrange("(ht p) t -> p ht t", p=128))
                ql8_t = qp.tile([128, HT, T], F8, tag="ql8", bufs=1)
                nc.sync.dma_start(
                    ql8_t[:], ql8_d[b].rearrange("(ht p) t -> p ht t", p=128))
                return qh_t, qh8_t, ql8_t

            q_next = (qh_first, qh8_first, ql8_first)

            for b in range(BL):
                qh_t, qh8_t, ql8_t = q_next
                eh_t = ep.tile([128, HT, S], F16, tag="eh")
                nc.sync.dma_start(
                    eh_t[:], eh_d[b].rearrange("(it p) s -> p it s", p=128))
                eh8_t = ep.tile([128, HT, S], F8, tag="eh8")
                nc.sync.dma_start(
                    eh8_t[:], eh8_d[b].rearrange("(it p) s -> p it s", p=128))
                el8_t = ep.tile([128, HT, S], F8, tag="el8")
                nc.sync.dma_start(
                    el8_t[:], el8_d[b].rearrange("(it p) s -> p it s", p=128))
                en_t = enp.tile([128, ST, H], F16, tag="en")
                nc.sync.dma_start(
                    en_t[:], en_d[b].rearrange("(st p) k -> p st k", p=128))
                if wo_t is None:
                    wo_t = wp.tile([128, 2 * HT, H], F16)
                    nc.sync.dma_start(
                        wo_t[:], wo_d.rearrange("(kt p) h -> p kt h", p=128))

                # --- zT = W_inT.T @ qT: fp16 main + fp8 DR corrections,
                # staged through an f32 combine buffer. b0 runs pass-major
                # (mains need only wh+qh; corrections wait on fp8 DMAs);
                # later batches interleave so the eviction chains spread.
                comb_t = scrp.tile([128, HT, T], F32, tag="comb")

                def z_main(it):
                    zps = psmm.tile([128, T], F32, tag="mm")
                    for ht in range(HT):
                        nc.tensor.matmul(
                            zps[:],
                            wh_t[:, ht, it * 128:(it + 1) * 128],
                            qh_t[:, ht, :],
                            start=(ht == 0), stop=(ht == HT - 1))
                    nc.vector.tensor_copy(comb_t[:, it, :], zps[:])

                def corr_mms(out_ps, ops, it_lo, it_n, sc_lo, sc_n):
                    j = 0
                    for lhs, rhs in ops:
                        for k in range(HT // 2):
                            nc.tensor.matmul(
                                out_ps[:],
                                lhs[:, 2 * k:2 * k + 2, it_lo:it_lo + it_n],
                                rhs[:, 2 * k:2 * k + 2, sc_lo:sc_lo + sc_n],
                                start=(j == 0), stop=(j == HT - 1),
                                perf_mode=DR)
                            j += 1

                def z_corr(it):
                    zcorr = psc.tile([128, T], F32, tag="mmc")
                    corr_mms(zcorr, ((wl8_t, qh8_t), (wh8_t, ql8_t)),
                             it * 128, 128, 0, T)
                    comb = comb_t[:, it, :]
                    nc.vector.scalar_tensor_tensor(
                        out=comb, in0=zcorr[:],
                        scalar=1.0 / (SC_WH * SC_QL), in1=comb,
                        op0=mybir.AluOpType.mult, op1=mybir.AluOpType.add)
                    nc.vector.tensor_copy(zh_t[:, it, :], comb)
                    zl_tmp = scrp2.tile([128, T], F16, tag="zltmp")
                    nc.vector.tensor_sub(zl_tmp[:], comb, zh_t[:, it, :])
                    nc.scalar.activation(
                        out=zl8_t[:, it, :], in_=zl_tmp[:],
                        func=mybir.ActivationFunctionType.Copy, scale=SC_ZL)
                    nc.scalar.activation(
                        out=zh8_t[:, it, :], in_=zh_t[:, it, :],
                        func=mybir.ActivationFunctionType.Copy)

                zh_t = zp.tile([128, HT, T], F16, tag="zh")
                zh8_t = zp.tile([128, HT, T], F8, tag="zh8")
                zl8_t = zp.tile([128, HT, T], F8, tag="zl8")
                if b == 0:
                    for it in range(HT):
                        z_main(it)
                    for it in range(HT):
                        z_corr(it)
                else:
                    for it in range(HT):
                        z_main(it)
                        z_corr(it)

                # --- scores + softmax -> p; transposes interleaved ---
                pt_t = ptp.tile([128, ST, T], F16, tag="pt")
                p_tiles = []

                def transposes(tt):
                    for st in range(ST):
                        tps = pstr.tile([128, 128], F16, tag="tr")
                        nc.tensor.transpose(
                            tps[:], p_tiles[tt][:, st * 128:(st + 1) * 128],
                            ident[:])
                        nc.vector.tensor_copy(
                            pt_t[:, st, tt * 128:(tt + 1) * 128], tps[:])

                def scores_half(sc_t, tt, sc, nmax):
                    sps = psmm.tile([128, 512], F32, tag="mm")
                    for it in range(HT):
                        nc.tensor.matmul(
                            sps[:],
                            zh_t[:, it, tt * 128:(tt + 1) * 128],
                            eh_t[:, it, sc * 512:(sc + 1) * 512],
                            start=(it == 0), stop=(it == HT - 1))
                    scorr = psc.tile([128, 512], F32, tag="mmc")
                    corr_mms(scorr, ((zl8_t, eh8_t), (zh8_t, el8_t)),
                             tt * 128, 128, sc * 512, 512)
                    chunk = sc_t[:, sc * 512:(sc + 1) * 512]
                    nc.vector.tensor_copy(chunk, sps[:])
                    nc.vector.scalar_tensor_tensor(
                        out=chunk, in0=scorr[:],
                        scalar=1.0 / (SC_ZL * SC_EH), in1=chunk,
                        op0=mybir.AluOpType.mult,
                        op1=mybir.AluOpType.add)
                    # row-max of this half (negated) overlaps the next half
                    nc.vector.reduce_max(out=nmax[:], in_=chunk,
                                         axis=mybir.AxisListType.X,
                                         negate=True)

                def softmax(sc_t, nm0, nm1):
                    negmax = stp.tile([128, 1], F32, tag="nm")
                    nc.vector.tensor_tensor(
                        out=negmax[:], in0=nm0[:], in1=nm1[:],
                        op=mybir.AluOpType.min)
                    p_t = pp.tile([128, S], F16, tag="p")
                    ssum = stp.tile([128, 1], F32, tag="ss")
                    nc.scalar.activation(
                        out=p_t[:], in_=sc_t[:],
                        func=mybir.ActivationFunctionType.Exp,
                        bias=negmax[:], scale=1.0, accum_out=ssum[:])
                    rsum = stp.tile([128, 1], F32, tag="rs")
                    nc.vector.reciprocal(rsum[:], ssum[:])
                    nc.vector.tensor_scalar_mul(p_t[:], p_t[:], rsum[:])
                    p_tiles.append(p_t)

                for tt in range(TT):
                    sc_t = scp.tile([128, S], F32, tag="sc")
                    nm0 = stp.tile([128, 1], F32, tag="nm0")
                    nm1 = stp.tile([128, 1], F32, tag="nm1")
                    scores_half(sc_t, tt, 0, nm0)
                    scores_half(sc_t, tt, 1, nm1)
                    softmax(sc_t, nm0, nm1)
                    if tt > 0:
                        transposes(tt - 1)

                # prefill the q-part of the out projection for tt=0 into the
                # (now idle) correction PSUM banks; covers the last softmax
                # chain + transposes(3) latency with useful PE work.
                oq = []
                for hc in range(2):
                    ops = psc.tile([128, 512], F32, tag="mmc")
                    for ht in range(HT):
                        nc.tensor.matmul(
                            ops[:],
                            qh_t[:, ht, 0:128],
                            wo_t[:, HT + ht, hc * 512:(hc + 1) * 512],
                            start=(ht == 0), stop=False)
                    oq.append(ops)
                transposes(TT - 1)

                # prefetch next batch's q tensors on SP now, BEFORE the out
                # stores occupy the queue.
                if b + 1 < BL:
                    q_next = load_q(b + 1)

                # --- cT = enc_nat.T @ pT -> [k, t] f16 ---
                ct_t = ctp.tile([128, HT, T], F16, tag="ct")
                for kt in range(HT):
                    cps = psmm.tile([128, T], F32, tag="mm")
                    for st in range(ST):
                        nc.tensor.matmul(
                            cps[:],
                            en_t[:, st, kt * 128:(kt + 1) * 128],
                            pt_t[:, st, :],
                            start=(st == 0), stop=(st == ST - 1))
                    nc.vector.tensor_copy(ct_t[:, kt, :], cps[:])

                # --- out = tanh(cT.T @ WcT + qT.T @ WqT + b) ---
                for tt in range(TT):
                    for hc in range(2):
                        if tt == 0:
                            ops = oq[hc]
                        else:
                            ops = psmm.tile([128, 512], F32, tag="mm")
                            for ht in range(HT):
                                nc.tensor.matmul(
                                    ops[:],
                                    qh_t[:, ht, tt * 128:(tt + 1) * 128],
                                    wo_t[:, HT + ht, hc * 512:(hc + 1) * 512],
                                    start=(ht == 0), stop=False)
                        for kt in range(HT):
                            nc.tensor.matmul(
                                ops[:],
                                ct_t[:, kt, tt * 128:(tt + 1) * 128],
                                wo_t[:, kt, hc * 512:(hc + 1) * 512],
                                start=False, stop=(kt == HT - 1))
                        last = (b == BL - 1 and tt == TT - 1 and hc == 1)
                        for piece in range(2 if last else 1):
                            npc = 256 if last else 512
                            po = piece * 256
                            ost = op.tile([128, npc], F32, tag="os2" if last else "os")
                            if has_bias:
                                nc.vector.tensor_add(
                                    ost[:], ops[:, po:po + npc],
                                    bias_t[:, hc * 512 + po:hc * 512 + po + npc])
                                nc.scalar.activation(
                                    out=ost[:], in_=ost[:],
                                    func=mybir.ActivationFunctionType.Tanh)
                            else:
                                nc.scalar.activation(
                                    out=ost[:], in_=ops[:, po:po + npc],
                                    func=mybir.ActivationFunctionType.Tanh)
                            nc.sync.dma_start(
                                out_d[b, tt * 128:(tt + 1) * 128,
                                      hc * 512 + po:hc * 512 + po + npc],
                                ost[:])

    nc.compile()
    return nc


def _get_nc(has_bias):
    key = ("nc", has_bias)
    if key not in _CACHE:
        _CACHE[key] = _build(has_bias)
    return _CACHE[key]


def _split16(x):
    hi = x.astype(np.float16)
    lo = (x - hi.astype(np.float32)).astype(np.float32)
    return hi, lo


def _f8(x, scale):
    return (np.asarray(x, np.float32) * np.float32(scale)).astype(
        ml_dtypes.float8_e4m3)


def kernel(query, encoder_outputs, src_lengths, W_in, W_out, b_out):
    query = np.asarray(query, np.float32)
    enc = np.asarray(encoder_outputs, np.float32)
    W_in = np.asarray(W_in, np.float32)
    W_out = np.asarray(W_out, np.float32)
    b_out = np.asarray(b_out, np.float32)

    # host-side layout prep (transposes + fp16 hi/lo splits)
    qT = np.ascontiguousarray(query.transpose(0, 2, 1))        # [B, H, T]
    qh, ql = _split16(qT)
    encT = np.ascontiguousarray(enc.transpose(1, 2, 0))        # [B, H, S]
    eh, el = _split16(encT)
    en = np.ascontiguousarray(enc.transpose(1, 0, 2)).astype(np.float16)
    whf, wlf = _split16(np.ascontiguousarray(W_in.T))          # [H(h), H(i)]
    wo = np.ascontiguousarray(W_out.T).astype(np.float16)      # [2H, H]
    bias = np.ascontiguousarray(
        np.broadcast_to(b_out[None, :], (128, H)), np.float32)

    common = {
        "wh": whf, "wo": wo, "bias": bias,
        "wh8": _f8(whf.astype(np.float32), SC_WH),
        "wl8": _f8(wlf, SC_WL),
    }

    in_maps = []
    for c in range(NCORES):
        sl = slice(c * BL, (c + 1) * BL)
        m = {
            "qh": np.ascontiguousarray(qh[sl]),
            "eh": np.ascontiguousarray(eh[sl]),
            "en": np.ascontiguousarray(en[sl]),
            "qh8": _f8(qh[sl].astype(np.float32), SC_QH),
            "ql8": _f8(ql[sl], SC_QL),
            "eh8": _f8(eh[sl].astype(np.float32), SC_EH),
            "el8": _f8(el[sl], SC_EL),
            **common,
        }
        in_maps.append(m)

    nc = _get_nc(bool(np.any(b_out)))
    trace = bool(int(os.environ.get("KERNEL_TRACE", "0")))
    res = run_bass_kernel_spmd(nc, in_maps, core_ids=list(range(NCORES)),
                               trace=trace)
    if trace:
        _CACHE["last_exec_time_ns"] = res.exec_time_ns
        _CACHE["last_results"] = res
    out = np.concatenate([r["out"] for r in res.results], axis=0)
    return out
